# revision 17
# baseline (speedup 1.0000x reference)
"""Trainium2 Bass kernel for CarlosSelfAttention (B=2, T=2048, C=1024, H=16).

Sharding: tensor-parallel over heads. 8 cores x 2 heads each.
Each core computes q/k/v projections for its 2 heads, RoPE, causal
attention, and a partial out-projection against its 128 columns of Wo.
The host sums the 8 partial outputs (the TP all-reduce) and adds the
output bias plus the (v-bias @ Wo.T) correction term.

v4: keep every engine busy through the phase boundaries.
  - PE pre-warm: junk matmuls during the initial input DMA so HAM is
    at K=8/8 when the first real matmul issues; exp table pre-loaded
    by a dummy ACTIVATE at t=0.
  - phase order: qkv(b0) -> [attn(b0) with deferred qkv(b1)+rope(b1)
    +v(b1) units flushed into the kt loop] -> attn(b1). The PE never
    sees a >1us idle gap, so HAM never re-throttles.
  - scalar engine carries only exp during attention; v-projection
    evictions go to scalar only in the qkv phase (where it is idle);
    diag-mask muls and rope(b1) run on gpsimd; psy evictions on DVE.
  - normalization without DMAs: psO_h0 keeps sum(exp) in row 64,
    psO_h1 gets it in row 0 (128-wide stationary: ones col 0, dims in
    cols 64:128), so both heads' reciprocals broadcast from their own
    psum rows into a full [128, q] tile and the muls write OT's two
    partition halves directly (no rz / otmp partition-move DMAs).
"""

import numpy as np
import ml_dtypes

import concourse.bass as bass
import concourse.tile as tile
from concourse import bacc, mybir
from concourse.bass_utils import run_bass_kernel_spmd

F32 = mybir.dt.float32
BF16 = mybir.dt.bfloat16
AF = mybir.ActivationFunctionType
NPBF16 = ml_dtypes.bfloat16

B, T, C, H, HD = 2, 2048, 1024, 16, 64
NCORES = 8
TB = B * T          # 4096
QCH = 512           # q-chunk (moving dim)
NQC = T // QCH      # 4 q-chunks per batch
NKT = T // 128      # 16 k-tiles per batch
NCT = C // 128      # 8 contraction tiles
NWARM = 10          # junk matmuls to pre-warm the PE clock gate

_PROG_CACHE: dict = {}


def _emit(tc, mode, hasb, dram):
    nc = tc.nc
    from contextlib import ExitStack

    wT, bqk, cosT, sinS, woT, y = (
        dram["wT"], dram["bqk"], dram["cosT"], dram["sinS"],
        dram["woT"], dram["y"])
    xT = dram["xT"]
    maskT = dram.get("maskT")

    with ExitStack() as ctx:
        constp = ctx.enter_context(tc.tile_pool(name="const", bufs=1))
        pers = ctx.enter_context(tc.tile_pool(name="pers", bufs=1))

        # ---- PE pre-warm + ACT exp-table preload (overlaps input DMA) ----
        junk = constp.tile([128, 512], BF16)
        nc.vector.memset(junk[:], 0.0)
        jexp = constp.tile([1, 8], F32)
        nc.scalar.activation(jexp[:], junk[0:1, 0:8], AF.Exp)
        with tc.tile_pool(name="warm", bufs=1, space="PSUM") as WP:
            jp = WP.tile([128, 256], F32)
            for _ in range(NWARM):
                nc.tensor.matmul(jp[:], junk[:, 0:128], junk[:, 0:256],
                                 start=True, stop=True)

        # ---- constants, in first-use order ----
        wsb = constp.tile([128, NCT, 384], BF16)
        nc.sync.dma_start(wsb[:, 0, :], wT[0:128, :])
        xq = [constp.tile([128, NCT, 1024], BF16, name=f"xq{i}")
              for i in range(4)]

        def xs(ct, lo, n):
            """x slice [128, n] covering global token cols [lo, lo+n)."""
            q, l = lo // 1024, lo % 1024
            assert l + n <= 1024
            return xq[q][:, ct, l:l + n]

        nc.sync.dma_start(
            wsb[:, 1:NCT, :],
            wT[128:NCT * 128, :].rearrange("(a p) m -> p a m", p=128))
        for ct in range(NCT):
            nc.sync.dma_start(xq[0][:, ct, :],
                              xT[ct * 128:(ct + 1) * 128, 0:1024])
        nc.sync.dma_start(
            xq[1][:], xT[:, 1024:2048].rearrange("(a p) m -> p a m", p=128))
        cos_sb = constp.tile([128, T], BF16)
        nc.sync.dma_start(cos_sb[:], cosT[:])
        sin_sb = constp.tile([128, T], BF16)
        nc.sync.dma_start(sin_sb[:], sinS[:])
        if hasb:
            bqk_sb = constp.tile([128, 2], F32)
            nc.sync.dma_start(bqk_sb[:], bqk[:])
        wo_sb = constp.tile([128, C], BF16)
        # b1's x quarters + wo are loaded later (after b0's qkv units are
        # emitted) so they don't steal HBM bandwidth from the critical
        # first tiles
        # selector column for the h0 denominator hop: a K=1 matmul with
        # this as stationary moves row 64 to partition 0 of a PSUM tile
        sel = constp.tile([128, 1], BF16)
        nc.vector.memset(sel[:], 1.0)
        # triangular causal mask for the diagonal 128-blocks, both heads:
        # [128, 2, 128], keep (1.0) iff col >= partition
        mtri = constp.tile([128, 2, 128], BF16)
        nc.vector.memset(mtri[:], 1.0)
        nc.gpsimd.affine_select(
            out=mtri[:], in_=mtri[:], compare_op=mybir.AluOpType.is_ge,
            fill=0.0, base=0, channel_multiplier=-1,
            pattern=[[0, 2], [1, 128]])

        # ---- persistent activations ----
        qT = pers.tile([128, TB], BF16)
        kT = pers.tile([128, TB], BF16)
        # V for head 0 in [ktok, dim] layout; col 64 of each k-tile stays
        # 1.0 so the P@V matmul also produces sum(exp) in psO row 64.
        # V for head 1 is 128 wide: ones in col 0 (sum(exp) -> psO row 0),
        # zeros in 1:64, dims in 64:128 (lands on OT partitions 64:128).
        V0 = [pers.tile([128, NKT * (HD + 1)], BF16, name=f"V0b{b}")
              for b in range(B)]
        V1 = [pers.tile([128, NKT, 128], BF16, name=f"V1b{b}")
              for b in range(B)]
        for b in range(B):
            nc.vector.memset(V0[b][:], 1.0)
            nc.vector.memset(V1[b][:], 0.0)
            nc.vector.memset(V1[b][:, :, 0:1], 1.0)
        OT = [pers.tile([128, T], BF16, name=f"OTb{b}") for b in range(B)]

        def qk_unit(P1, tca, tcb, g):
            """q (g=0) or k (g=1) projection for two adjacent 512-token
            chunks, shared stationaries, one [128, 1024] PSUM tile."""
            ts2 = slice(tca * QCH, (tcb + 1) * QCH)
            psab = P1.tile([128, 2 * QCH], F32, tag="s",
                           name=f"psq{tca}_{g}")
            for ct in range(NCT):
                w = wsb[:, ct, g * 128:(g + 1) * 128]
                nc.tensor.matmul(psab[:, 0:QCH], w,
                                 xs(ct, tca * QCH, QCH),
                                 start=(ct == 0), stop=(ct == NCT - 1))
                nc.tensor.matmul(psab[:, QCH:2 * QCH], w,
                                 xs(ct, tcb * QCH, QCH),
                                 start=(ct == 0), stop=(ct == NCT - 1))
            dst = qT if g == 0 else kT
            if hasb:
                nc.scalar.activation(dst[:, ts2], psab[:], AF.Identity,
                                     bias=bqk_sb[:, g:g + 1])
            else:
                nc.vector.tensor_copy(dst[:, ts2], psab[:])

        def v_unit(P2, b, tt0, ntt, evict):
            """V tiles [tt0, tt0+ntt) for batch b in [ktok, dim] layout:
            stationary is the x tile, moving is the 128 Wv columns."""
            for tt in range(tt0, tt0 + ntt):
                lo = b * T + tt * 128
                psv = P2.tile([128, 128], F32, tag=f"o{tt % 2}",
                              name=f"psv{b}{tt}")
                for ct in range(NCT):
                    nc.tensor.matmul(psv[:], xs(ct, lo, 128),
                                     wsb[:, ct, 256:384],
                                     start=(ct == 0), stop=(ct == NCT - 1))
                o = tt * (HD + 1)
                if evict == "scalar":
                    nc.scalar.activation(V0[b][:, o:o + HD], psv[:, 0:HD],
                                         AF.Copy)
                    nc.scalar.activation(V1[b][:, tt, 64:128],
                                         psv[:, HD:2 * HD], AF.Copy)
                else:
                    nc.vector.tensor_copy(V0[b][:, o:o + HD], psv[:, 0:HD])
                    nc.vector.tensor_copy(V1[b][:, tt, 64:128],
                                          psv[:, HD:2 * HD])

        def rope_p(swpp, rtp, zt, b, piece, nm):
            """RoPE for one 1024-token piece (chunk pair) of batch b, so
            attention can start before the whole batch is rotated."""
            bs = slice(b * T + piece * 1024, b * T + (piece + 1) * 1024)
            cs = slice(piece * 1024, (piece + 1) * 1024)
            swp = swpp.tile([128, 1024], BF16, tag="swp", name=f"swp{nm}")
            for h in range(2):
                o = h * 64
                nc.sync.dma_start(swp[o:o + 32, :], zt[o + 32:o + 64, bs])
                nc.sync.dma_start(swp[o + 32:o + 64, :], zt[o:o + 32, bs])
            tmp = rtp.tile([128, 1024], BF16, tag="rt", name=f"rt{nm}")
            nc.vector.tensor_mul(tmp[:], swp[:], sin_sb[:, cs])
            nc.vector.tensor_mul(zt[:, bs], zt[:, bs], cos_sb[:, cs])
            nc.vector.tensor_add(zt[:, bs], zt[:, bs], tmp[:])

        def proj_qc(pso, ybn, b, qc, eng_pick):
            """Out-projection units for the 4 token-tiles of (b, qc)."""
            for tt in range(4 * qc, 4 * qc + 4):
                yb = ybn.tile([128, C], BF16, tag="yb", name=f"yb{b}{tt}")
                for ncol in range(2):
                    ps = pso.tile([128, QCH], F32, tag=f"o{ncol}",
                                  name=f"psy{b}{tt}{ncol}")
                    nc.tensor.matmul(
                        ps[:], OT[b][:, tt * 128:(tt + 1) * 128],
                        wo_sb[:, ncol * QCH:(ncol + 1) * QCH],
                        start=True, stop=True)
                    ysl = yb[:, ncol * QCH:(ncol + 1) * QCH]
                    if eng_pick(tt, ncol) == "scalar":
                        nc.scalar.activation(ysl, ps[:], AF.Copy)
                    else:
                        nc.vector.tensor_copy(ysl, ps[:])
                nc.sync.dma_start(
                    y[b * T + tt * 128:b * T + (tt + 1) * 128, :], yb[:])

        def attn_b(pools, b, ybn, pending, filler, eng_pick):
            pss, pso, ptp, mbp, smol = pools
            PIPE = 5
            for qc in range(NQC):
                nk = 4 * (qc + 1) if mode == "causal" else NKT
                psO0 = pso.tile([65, QCH], F32, tag="o0",
                                name=f"psO0_{b}{qc}")
                psO1 = pso.tile([128, QCH], F32, tag="o1",
                                name=f"psO1_{b}{qc}")
                pts = {}

                def offs(kt):
                    if mode == "causal" and kt >= 4 * qc:
                        return (kt - 4 * qc) * 128
                    return 0

                def emit_pv(j, nk=nk, psO0=psO0, psO1=psO1, pts=pts, b=b):
                    st, sp = (j == 0), (j == nk - 1)
                    off = offs(j)
                    pt = pts.pop(j)
                    ptv = pt[:].rearrange("p (h q) -> p h q", q=QCH)
                    nc.tensor.matmul(
                        psO0[:, off:QCH],
                        V0[b][:, j * (HD + 1):(j + 1) * (HD + 1)],
                        ptv[:, 0, off:QCH], start=st, stop=sp)
                    nc.tensor.matmul(
                        psO1[:, off:QCH], V1[b][:, j, :],
                        ptv[:, 1, off:QCH], start=st, stop=sp)

                for kt in range(nk):
                    if kt == 1 and len(pending) >= 2:
                        # flush an out-projection deferred by two q-chunks
                        pending.pop(0)()
                    if kt >= 2 and filler:
                        # flush one deferred unit of the other batch's
                        # qkv/rope work into this kt slot
                        filler.pop(0)()
                    ks = slice(b * T + kt * 128, b * T + (kt + 1) * 128)
                    off = offs(kt)
                    qs = slice(b * T + qc * QCH + off, b * T + (qc + 1) * QCH)
                    psS = pss.tile([128, 2 * QCH], F32, tag="s",
                                   name=f"psS{b}{qc}{kt}")
                    # the two heads occupy disjoint PE row groups
                    # (partitions 0:64 / 64:128) -> emitted back-to-back
                    # they execute concurrently
                    nc.tensor.matmul(psS[:, off:QCH], kT[0:64, ks],
                                     qT[0:64, qs], start=True, stop=True)
                    nc.tensor.matmul(psS[:, QCH + off:2 * QCH],
                                     kT[64:128, ks], qT[64:128, qs],
                                     start=True, stop=True)
                    pt = ptp.tile([128, 2 * QCH], BF16, tag="pt",
                                  name=f"pt{b}{qc}{kt}")
                    psv_ = psS[:].rearrange("p (h q) -> p h q", q=QCH)
                    ptv = pt[:].rearrange("p (h q) -> p h q", q=QCH)
                    nc.scalar.activation(ptv[:, 0, off:QCH],
                                         psv_[:, 0, off:QCH], AF.Exp)
                    nc.scalar.activation(ptv[:, 1, off:QCH],
                                         psv_[:, 1, off:QCH], AF.Exp)
                    if mode == "causal" and kt >= 4 * qc:
                        nc.vector.tensor_mul(ptv[:, :, off:off + 128],
                                             ptv[:, :, off:off + 128],
                                             mtri[:])
                    elif mode == "bias":
                        mt = mbp.tile([128, QCH], BF16, tag="mb",
                                      name=f"mt{b}{qc}{kt}")
                        nc.sync.dma_start(
                            mt[:], maskT[kt * 128:(kt + 1) * 128,
                                         qc * QCH:(qc + 1) * QCH])
                        nc.vector.tensor_mul(ptv[:, 0, :], ptv[:, 0, :],
                                             mt[:])
                        nc.vector.tensor_mul(ptv[:, 1, :], ptv[:, 1, :],
                                             mt[:])
                    pts[kt] = pt
                    if kt >= PIPE:
                        emit_pv(kt - PIPE)
                for j in range(max(0, nk - PIPE), nk):
                    emit_pv(j)

                # normalize + evict; sum(exp) is in psO0 row 64 / psO1
                # row 0. First hop copies psO out of PSUM so the bank
                # frees for the next q-chunk's P@V; reciprocal of the sum
                # row broadcasts (gpsimd, partition 0 of the src AP) into
                # a full [128, q] tile, and the muls write OT's partition
                # halves in place -- no partition-moving DMAs.
                oqs = slice(qc * QCH, (qc + 1) * QCH)
                nm = f"{b}{qc}"
                # h1's sum(exp) is already at psO1 partition 0; h0's sits
                # at psO0 row 64 -- hop it to partition 0 of a PSUM tile
                # via a K=1 matmul so partition_broadcast can read it.
                dcp = smol.tile([65, QCH], BF16, tag="dcp", name=f"dcp{nm}")
                nc.vector.tensor_copy(dcp[64:65, :], psO0[64:65, :])
                rb1 = smol.tile([1, QCH], F32, tag="rb1", name=f"rb1{nm}")
                nc.vector.reciprocal_approx_fast(rb1[:], psO1[0:1, :])
                sden = pso.tile([1, QCH], F32, tag="o0", name=f"sden{nm}")
                nc.tensor.matmul(sden[:], sel[64:65, :], dcp[64:65, :],
                                 start=True, stop=True)
                rb0 = smol.tile([1, QCH], F32, tag="rb0", name=f"rb0{nm}")
                nc.vector.reciprocal_approx_fast(rb0[:], sden[:])
                bc1 = smol.tile([128, QCH], F32, tag="bc1", name=f"bc1{nm}")
                nc.gpsimd.partition_broadcast(bc1[:], rb1[:])
                bc0 = smol.tile([128, QCH], F32, tag="bc0", name=f"bc0{nm}")
                nc.gpsimd.partition_broadcast(bc0[:], rb0[:])
                nc.vector.tensor_mul(OT[b][64:128, oqs], psO1[64:128, :],
                                     bc1[64:128, :])
                nc.vector.tensor_mul(OT[b][0:64, oqs], psO0[0:64, :],
                                     bc0[0:64, :])

                pending.append(
                    lambda b=b, qc=qc: proj_qc(pso, ybn, b, qc, eng_pick))

        # ---- phases ----
        with tc.tile_pool(name="P1", bufs=2, space="PSUM") as P1, \
             tc.tile_pool(name="P2", bufs=2, space="PSUM") as P2, \
             tc.tile_pool(name="swp", bufs=2) as swpp, \
             tc.tile_pool(name="rtmp", bufs=2) as rtp, \
             tc.tile_pool(name="ptp", bufs=6) as ptp, \
             tc.tile_pool(name="mbp", bufs=4) as mbp, \
             tc.tile_pool(name="smol", bufs=2) as smol, \
             tc.tile_pool(name="ybp", bufs=4) as ybp:
            pending = []
            psy_n = [0]

            def eng_pick(tt, ncol):
                psy_n[0] += 1
                return "scalar" if psy_n[0] % 6 == 5 else "vector"

            # minimal prereqs for attn(b0) qc0: q/k of tokens 0:1024
            # roped + V tiles 0:4; everything else streams in as filler.
            qk_unit(P1, 0, 1, 0)
            qk_unit(P1, 0, 1, 1)
            rope_p(swpp, rtp, qT, 0, 0, "q00")
            rope_p(swpp, rtp, kT, 0, 0, "k00")
            # b1's inputs + wo stream in later: the marker memsets gate
            # the DMAs (WAR) so they don't steal HBM bandwidth from the
            # critical lead-in tiles
            nc.vector.memset(xq[2][0:1, 0, 0:1], 0.0)
            nc.vector.memset(xq[3][0:1, 0, 0:1], 0.0)
            nc.vector.memset(wo_sb[0:1, 0:1], 0.0)
            nc.sync.dma_start(
                xq[2][:],
                xT[:, 2048:3072].rearrange("(a p) m -> p a m", p=128))
            nc.sync.dma_start(
                xq[3][:],
                xT[:, 3072:4096].rearrange("(a p) m -> p a m", p=128))
            nc.sync.dma_start(wo_sb[:], woT[:])
            v_unit(P2, 0, 0, 4, "scalar")

            # deferred work flushed inside attn(b0)'s kt loop: rest of
            # b0's qkv/rope, then b1's qkv + rope
            filler = [
                lambda: qk_unit(P1, 2, 3, 0),
                lambda: v_unit(P2, 0, 4, 4, "scalar"),
                lambda: qk_unit(P1, 2, 3, 1),
                lambda: rope_p(swpp, rtp, qT, 0, 1, "q01"),
                lambda: rope_p(swpp, rtp, kT, 0, 1, "k01"),
                lambda: v_unit(P2, 0, 8, 4, "vector"),
                lambda: v_unit(P2, 0, 12, 4, "vector"),
                lambda: qk_unit(P1, 4, 5, 0),
                lambda: qk_unit(P1, 4, 5, 1),
                lambda: rope_p(swpp, rtp, qT, 1, 0, "q10"),
                lambda: rope_p(swpp, rtp, kT, 1, 0, "k10"),
                lambda: qk_unit(P1, 6, 7, 0),
                lambda: qk_unit(P1, 6, 7, 1),
                lambda: rope_p(swpp, rtp, qT, 1, 1, "q11"),
                lambda: rope_p(swpp, rtp, kT, 1, 1, "k11"),
            ]

            attn_b((P1, P2, ptp, mbp, smol), 0, ybp, pending, filler,
                   eng_pick)
            # b1's V units flush inside attn(b1): tiles 0-7 are needed
            # from qc0/qc1, emit them first
            filler2 = [
                lambda u=u: v_unit(P2, 1, 4 * u, 4, "vector")
                for u in range(4)
            ]
            for f in filler:
                f()
            attn_b((P1, P2, ptp, mbp, smol), 1, ybp, pending, filler2,
                   eng_pick)
            for f in filler2:
                f()
            for p in pending:
                p()


def _build_program(mode, hasb):
    key = (mode, hasb)
    if key in _PROG_CACHE:
        return _PROG_CACHE[key]
    nc = bacc.Bacc("TRN2", target_bir_lowering=False, debug=False,
                   num_devices=NCORES)
    dram = {
        "xT": nc.dram_tensor("xT", [C, TB], BF16, kind="ExternalInput").ap(),
        "wT": nc.dram_tensor("wT", [C, 384], BF16, kind="ExternalInput").ap(),
        "bqk": nc.dram_tensor("bqk", [128, 2], F32, kind="ExternalInput").ap(),
        "cosT": nc.dram_tensor("cosT", [128, T], BF16,
                               kind="ExternalInput").ap(),
        "sinS": nc.dram_tensor("sinS", [128, T], BF16,
                               kind="ExternalInput").ap(),
        "woT": nc.dram_tensor("woT", [128, C], BF16,
                              kind="ExternalInput").ap(),
        "y": nc.dram_tensor("y", [TB, C], BF16, kind="ExternalOutput").ap(),
    }
    if mode == "bias":
        dram["maskT"] = nc.dram_tensor("maskT", [T, T], BF16,
                                       kind="ExternalInput").ap()
    with tile.TileContext(nc) as tc:
        _emit(tc, mode, hasb, dram)
    nc.compile()
    _PROG_CACHE[key] = (nc, dram)
    return nc, dram


def _rope_tables():
    inv_freq = 1.0 / (10000.0 ** (np.arange(0, HD, 2, dtype=np.float64) / HD))
    freqs = np.arange(T, dtype=np.float64)[:, None] * inv_freq[None, :]
    cos = np.concatenate([np.cos(freqs), np.cos(freqs)], axis=-1)  # [T, 64]
    sin = np.concatenate([np.sin(freqs), np.sin(freqs)], axis=-1)
    cE = cos[:, 0::2].T  # [32, T] rows i -> dim 2i
    cO = cos[:, 1::2].T
    sE = sin[:, 0::2].T
    sO = sin[:, 1::2].T
    cosT = np.concatenate([cE, cO, cE, cO], axis=0)
    sinS = np.concatenate([-sE, sO, -sE, sO], axis=0)
    return (np.ascontiguousarray(cosT.astype(NPBF16)),
            np.ascontiguousarray(sinS.astype(NPBF16)))


def _detect_mode(mask):
    mb = mask.reshape(T, T)
    if np.array_equal(mb != 0, np.tril(np.ones((T, T), dtype=bool))):
        return "causal"
    if np.all(mb != 0):
        return "dense"
    return "bias"


def _prepare_in_maps(x, mask, Wqkv, bqkv, Wo, mode):
    xTn = np.ascontiguousarray(
        x.reshape(TB, C).T.astype(NPBF16))
    cosT, sinS = _rope_tables()
    scale = 1.0 / np.sqrt(np.float32(HD))

    evens = np.arange(0, HD, 2)
    odds = evens + 1

    in_maps = []
    for c in range(NCORES):
        h0, h1 = 2 * c, 2 * c + 1
        qrows = np.concatenate([h0 * HD + evens, h0 * HD + odds,
                                h1 * HD + evens, h1 * HD + odds])
        krows = C + qrows
        vrows = np.concatenate([2 * C + h0 * HD + np.arange(HD),
                                2 * C + h1 * HD + np.arange(HD)])
        wq = Wqkv[qrows, :] * scale
        wk = Wqkv[krows, :]
        wv = Wqkv[vrows, :]
        wT = np.ascontiguousarray(
            np.concatenate([wq, wk, wv], axis=0).T.astype(NPBF16))
        bqk = np.stack([bqkv[qrows] * scale, bqkv[krows]], axis=1)
        woT = np.ascontiguousarray(
            Wo[:, 128 * c:128 * (c + 1)].T.astype(NPBF16))
        im = {
            "xT": xTn, "wT": wT,
            "bqk": np.ascontiguousarray(bqk, dtype=np.float32),
            "cosT": cosT, "sinS": sinS, "woT": woT,
        }
        if mode == "bias":
            im["maskT"] = np.ascontiguousarray(
                (mask.reshape(T, T) != 0).T.astype(NPBF16))
        in_maps.append(im)
    return in_maps


def kernel(x, mask, Wqkv, bqkv, Wo, bo):
    x = np.asarray(x, dtype=np.float32)
    mask = np.asarray(mask)
    Wqkv = np.asarray(Wqkv, dtype=np.float32)
    bqkv = np.asarray(bqkv, dtype=np.float32)
    Wo = np.asarray(Wo, dtype=np.float32)
    bo = np.asarray(bo, dtype=np.float32)

    mode = _detect_mode(mask)
    hasb = bool(np.any(bqkv[:2 * C] != 0.0))
    nc, dram = _build_program(mode, hasb)
    in_maps = _prepare_in_maps(x, mask, Wqkv, bqkv, Wo, mode)

    res = run_bass_kernel_spmd(nc, in_maps, core_ids=list(range(NCORES)))
    y = np.zeros((TB, C), dtype=np.float32)
    for c in range(NCORES):
        y += res.results[c]["y"].astype(np.float32)
    bv = bqkv[2 * C:3 * C]
    y += (bo + bv @ Wo.T)[None, :]
    return y.reshape(B, T, C)


# revision 18
# speedup vs baseline: 1.0613x; 1.0613x over previous
"""Trainium2 Bass kernel for CarlosSelfAttention (B=2, T=2048, C=1024, H=16).

Sharding: tensor-parallel over heads. 8 cores x 2 heads each.
Each core computes q/k/v projections for its 2 heads, RoPE, causal
attention, and a partial out-projection against its 128 columns of Wo.
The host sums the 8 partial outputs (the TP all-reduce) and adds the
output bias plus the (v-bias @ Wo.T) correction term.

v4: keep every engine busy through the phase boundaries.
  - PE pre-warm: junk matmuls during the initial input DMA so HAM is
    at K=8/8 when the first real matmul issues; exp table pre-loaded
    by a dummy ACTIVATE at t=0.
  - phase order: qkv(b0) -> [attn(b0) with deferred qkv(b1)+rope(b1)
    +v(b1) units flushed into the kt loop] -> attn(b1). The PE never
    sees a >1us idle gap, so HAM never re-throttles.
  - scalar engine carries only exp during attention; v-projection
    evictions go to scalar only in the qkv phase (where it is idle);
    diag-mask muls and rope(b1) run on gpsimd; psy evictions on DVE.
  - normalization without DMAs: psO_h0 keeps sum(exp) in row 64,
    psO_h1 gets it in row 0 (128-wide stationary: ones col 0, dims in
    cols 64:128), so both heads' reciprocals broadcast from their own
    psum rows into a full [128, q] tile and the muls write OT's two
    partition halves directly (no rz / otmp partition-move DMAs).
"""

import numpy as np
import ml_dtypes

import concourse.bass as bass
import concourse.tile as tile
from concourse import bacc, mybir
from concourse.bass_utils import run_bass_kernel_spmd

F32 = mybir.dt.float32
BF16 = mybir.dt.bfloat16
AF = mybir.ActivationFunctionType
NPBF16 = ml_dtypes.bfloat16

B, T, C, H, HD = 2, 2048, 1024, 16, 64
NCORES = 8
TB = B * T          # 4096
QCH = 512           # q-chunk (moving dim)
NQC = T // QCH      # 4 q-chunks per batch
NKT = T // 128      # 16 k-tiles per batch
NCT = C // 128      # 8 contraction tiles
NWARM = 10          # junk matmuls to pre-warm the PE clock gate

_PROG_CACHE: dict = {}


def _emit(tc, mode, hasb, dram):
    nc = tc.nc
    from contextlib import ExitStack

    wT, bqk, cosT, sinS, woT, y = (
        dram["wT"], dram["bqk"], dram["cosT"], dram["sinS"],
        dram["woT"], dram["y"])
    xT = dram["xT"]
    maskT = dram.get("maskT")

    with ExitStack() as ctx:
        constp = ctx.enter_context(tc.tile_pool(name="const", bufs=1))
        pers = ctx.enter_context(tc.tile_pool(name="pers", bufs=1))

        # ---- PE pre-warm + ACT exp-table preload (overlaps input DMA) ----
        junk = constp.tile([128, 512], BF16)
        nc.vector.memset(junk[:], 0.0)
        jexp = constp.tile([1, 8], F32)
        nc.scalar.activation(jexp[:], junk[0:1, 0:8], AF.Exp)
        with tc.tile_pool(name="warm", bufs=1, space="PSUM") as WP:
            jp = WP.tile([128, 256], F32)
            for _ in range(NWARM):
                nc.tensor.matmul(jp[:], junk[:, 0:128], junk[:, 0:256],
                                 start=True, stop=True)

        # ---- constants, in first-use order ----
        wsb = constp.tile([128, NCT, 384], BF16)
        nc.sync.dma_start(wsb[:, 0, :], wT[0:128, :])
        xq = [constp.tile([128, NCT, 1024], BF16, name=f"xq{i}")
              for i in range(4)]

        def xs(ct, lo, n):
            """x slice [128, n] covering global token cols [lo, lo+n)."""
            q, l = lo // 1024, lo % 1024
            assert l + n <= 1024
            return xq[q][:, ct, l:l + n]

        nc.sync.dma_start(
            wsb[:, 1:NCT, :],
            wT[128:NCT * 128, :].rearrange("(a p) m -> p a m", p=128))
        for ct in range(NCT):
            nc.sync.dma_start(xq[0][:, ct, :],
                              xT[ct * 128:(ct + 1) * 128, 0:1024])
        nc.sync.dma_start(
            xq[1][:], xT[:, 1024:2048].rearrange("(a p) m -> p a m", p=128))
        cos_sb = constp.tile([128, T], BF16)
        nc.sync.dma_start(cos_sb[:], cosT[:])
        sin_sb = constp.tile([128, T], BF16)
        nc.sync.dma_start(sin_sb[:], sinS[:])
        if hasb:
            bqk_sb = constp.tile([128, 2], F32)
            nc.sync.dma_start(bqk_sb[:], bqk[:])
        wo_sb = constp.tile([128, C], BF16)
        # b1's x quarters + wo are loaded later (after b0's qkv units are
        # emitted) so they don't steal HBM bandwidth from the critical
        # first tiles
        # selector column for the h0 denominator hop: a K=1 matmul with
        # this as stationary moves row 64 to partition 0 of a PSUM tile
        sel = constp.tile([128, 1], BF16)
        nc.vector.memset(sel[:], 1.0)
        # triangular causal mask for the diagonal 128-blocks, both heads:
        # [128, 2, 128], keep (1.0) iff col >= partition
        mtri = constp.tile([128, 2, 128], BF16)
        nc.vector.memset(mtri[:], 1.0)
        nc.gpsimd.affine_select(
            out=mtri[:], in_=mtri[:], compare_op=mybir.AluOpType.is_ge,
            fill=0.0, base=0, channel_multiplier=-1,
            pattern=[[0, 2], [1, 128]])

        # ---- persistent activations ----
        qT = pers.tile([128, TB], BF16)
        kT = pers.tile([128, TB], BF16)
        # V for head 0 in [ktok, dim] layout; col 64 of each k-tile stays
        # 1.0 so the P@V matmul also produces sum(exp) in psO row 64.
        # V for head 1 is 128 wide: ones in col 0 (sum(exp) -> psO row 0),
        # zeros in 1:64, dims in 64:128 (lands on OT partitions 64:128).
        V0 = [pers.tile([128, NKT * (HD + 1)], BF16, name=f"V0b{b}")
              for b in range(B)]
        V1 = [pers.tile([128, NKT, 128], BF16, name=f"V1b{b}")
              for b in range(B)]
        for b in range(B):
            nc.vector.memset(V0[b][:], 1.0)
            nc.vector.memset(V1[b][:], 0.0)
            nc.vector.memset(V1[b][:, :, 0:1], 1.0)
        OT = [pers.tile([128, T], BF16, name=f"OTb{b}") for b in range(B)]

        def qk_unit(P1, tca, tcb, g):
            """q (g=0) or k (g=1) projection for two adjacent 512-token
            chunks, shared stationaries, one [128, 1024] PSUM tile."""
            ts2 = slice(tca * QCH, (tcb + 1) * QCH)
            psab = P1.tile([128, 2 * QCH], F32, tag="s",
                           name=f"psq{tca}_{g}")
            for ct in range(NCT):
                w = wsb[:, ct, g * 128:(g + 1) * 128]
                nc.tensor.matmul(psab[:, 0:QCH], w,
                                 xs(ct, tca * QCH, QCH),
                                 start=(ct == 0), stop=(ct == NCT - 1))
                nc.tensor.matmul(psab[:, QCH:2 * QCH], w,
                                 xs(ct, tcb * QCH, QCH),
                                 start=(ct == 0), stop=(ct == NCT - 1))
            dst = qT if g == 0 else kT
            if hasb:
                nc.scalar.activation(dst[:, ts2], psab[:], AF.Identity,
                                     bias=bqk_sb[:, g:g + 1])
            else:
                nc.vector.tensor_copy(dst[:, ts2], psab[:])

        def v_unit(P2, b, tt0, ntt, evict):
            """V tiles [tt0, tt0+ntt) for batch b in [ktok, dim] layout:
            stationary is the x tile, moving is the 128 Wv columns."""
            for tt in range(tt0, tt0 + ntt):
                lo = b * T + tt * 128
                psv = P2.tile([128, 128], F32, tag=f"o{tt % 2}",
                              name=f"psv{b}{tt}")
                for ct in range(NCT):
                    nc.tensor.matmul(psv[:], xs(ct, lo, 128),
                                     wsb[:, ct, 256:384],
                                     start=(ct == 0), stop=(ct == NCT - 1))
                o = tt * (HD + 1)
                if evict == "scalar":
                    nc.scalar.activation(V0[b][:, o:o + HD], psv[:, 0:HD],
                                         AF.Copy)
                    nc.scalar.activation(V1[b][:, tt, 64:128],
                                         psv[:, HD:2 * HD], AF.Copy)
                else:
                    nc.vector.tensor_copy(V0[b][:, o:o + HD], psv[:, 0:HD])
                    nc.vector.tensor_copy(V1[b][:, tt, 64:128],
                                          psv[:, HD:2 * HD])

        def rope_p(swpp, rtp, zt, b, piece, nm):
            """RoPE for one 1024-token piece (chunk pair) of batch b, so
            attention can start before the whole batch is rotated."""
            bs = slice(b * T + piece * 1024, b * T + (piece + 1) * 1024)
            cs = slice(piece * 1024, (piece + 1) * 1024)
            swp = swpp.tile([128, 1024], BF16, tag="swp", name=f"swp{nm}")
            for h in range(2):
                o = h * 64
                nc.sync.dma_start(swp[o:o + 32, :], zt[o + 32:o + 64, bs])
                nc.sync.dma_start(swp[o + 32:o + 64, :], zt[o:o + 32, bs])
            tmp = rtp.tile([128, 1024], BF16, tag="rt", name=f"rt{nm}")
            nc.vector.tensor_mul(tmp[:], swp[:], sin_sb[:, cs])
            nc.vector.tensor_mul(zt[:, bs], zt[:, bs], cos_sb[:, cs])
            nc.vector.tensor_add(zt[:, bs], zt[:, bs], tmp[:])

        def proj_qc(pso, ybn, b, qc, eng_pick):
            """Out-projection units for the 4 token-tiles of (b, qc)."""
            for tt in range(4 * qc, 4 * qc + 4):
                yb = ybn.tile([128, C], BF16, tag="yb", name=f"yb{b}{tt}")
                for ncol in range(2):
                    ps = pso.tile([128, QCH], F32, tag=f"o{ncol}",
                                  name=f"psy{b}{tt}{ncol}")
                    nc.tensor.matmul(
                        ps[:], OT[b][:, tt * 128:(tt + 1) * 128],
                        wo_sb[:, ncol * QCH:(ncol + 1) * QCH],
                        start=True, stop=True)
                    ysl = yb[:, ncol * QCH:(ncol + 1) * QCH]
                    if eng_pick(tt, ncol) == "scalar":
                        nc.scalar.activation(ysl, ps[:], AF.Copy)
                    else:
                        nc.vector.tensor_copy(ysl, ps[:])
                nc.sync.dma_start(
                    y[b * T + tt * 128:b * T + (tt + 1) * 128, :], yb[:])

        def attn_b(pools, b, ybn, pending, filler, eng_pick):
            pss, pso, ptp, mbp, smol = pools
            PIPE = 5
            for qc in range(NQC):
                nk = 4 * (qc + 1) if mode == "causal" else NKT
                psO0 = pso.tile([65, QCH], F32, tag="o0",
                                name=f"psO0_{b}{qc}")
                psO1 = pso.tile([128, QCH], F32, tag="o1",
                                name=f"psO1_{b}{qc}")
                pts = {}

                def offs(kt):
                    if mode == "causal" and kt >= 4 * qc:
                        return (kt - 4 * qc) * 128
                    return 0

                def emit_pv(j, nk=nk, psO0=psO0, psO1=psO1, pts=pts, b=b):
                    st, sp = (j == 0), (j == nk - 1)
                    off = offs(j)
                    pt = pts.pop(j)
                    ptv = pt[:].rearrange("p (h q) -> p h q", q=QCH)
                    nc.tensor.matmul(
                        psO0[:, off:QCH],
                        V0[b][:, j * (HD + 1):(j + 1) * (HD + 1)],
                        ptv[:, 0, off:QCH], start=st, stop=sp)
                    nc.tensor.matmul(
                        psO1[:, off:QCH], V1[b][:, j, :],
                        ptv[:, 1, off:QCH], start=st, stop=sp)

                for kt in range(nk):
                    if kt == 1 and len(pending) >= 2:
                        # flush an out-projection deferred by two q-chunks
                        pending.pop(0)()
                    if kt >= 2 and filler and (b, qc) != (0, 0):
                        # flush deferred qkv/rope/v units into this kt
                        # slot (not in qc0, whose data is still landing)
                        filler.pop(0)()
                        if len(filler) > 8:
                            filler.pop(0)()
                    ks = slice(b * T + kt * 128, b * T + (kt + 1) * 128)
                    off = offs(kt)
                    qs = slice(b * T + qc * QCH + off, b * T + (qc + 1) * QCH)
                    psS = pss.tile([128, 2 * QCH], F32, tag="s",
                                   name=f"psS{b}{qc}{kt}")
                    # the two heads occupy disjoint PE row groups
                    # (partitions 0:64 / 64:128) -> emitted back-to-back
                    # they execute concurrently
                    nc.tensor.matmul(psS[:, off:QCH], kT[0:64, ks],
                                     qT[0:64, qs], start=True, stop=True)
                    nc.tensor.matmul(psS[:, QCH + off:2 * QCH],
                                     kT[64:128, ks], qT[64:128, qs],
                                     start=True, stop=True)
                    pt = ptp.tile([128, 2 * QCH], BF16, tag="pt",
                                  name=f"pt{b}{qc}{kt}")
                    psv_ = psS[:].rearrange("p (h q) -> p h q", q=QCH)
                    ptv = pt[:].rearrange("p (h q) -> p h q", q=QCH)
                    nc.scalar.activation(ptv[:, :, off:QCH],
                                         psv_[:, :, off:QCH], AF.Exp)
                    if mode == "causal" and kt >= 4 * qc:
                        nc.vector.tensor_mul(ptv[:, :, off:off + 128],
                                             ptv[:, :, off:off + 128],
                                             mtri[:])
                    elif mode == "bias":
                        mt = mbp.tile([128, QCH], BF16, tag="mb",
                                      name=f"mt{b}{qc}{kt}")
                        nc.sync.dma_start(
                            mt[:], maskT[kt * 128:(kt + 1) * 128,
                                         qc * QCH:(qc + 1) * QCH])
                        nc.vector.tensor_mul(ptv[:, 0, :], ptv[:, 0, :],
                                             mt[:])
                        nc.vector.tensor_mul(ptv[:, 1, :], ptv[:, 1, :],
                                             mt[:])
                    pts[kt] = pt
                    if kt >= PIPE:
                        emit_pv(kt - PIPE)
                for j in range(max(0, nk - PIPE), nk):
                    emit_pv(j)

                # normalize + evict; sum(exp) is in psO0 row 64 / psO1
                # row 0. First hop copies psO out of PSUM so the bank
                # frees for the next q-chunk's P@V; reciprocal of the sum
                # row broadcasts (gpsimd, partition 0 of the src AP) into
                # a full [128, q] tile, and the muls write OT's partition
                # halves in place -- no partition-moving DMAs.
                oqs = slice(qc * QCH, (qc + 1) * QCH)
                nm = f"{b}{qc}"
                # h1's sum(exp) is already at psO1 partition 0; h0's sits
                # at psO0 row 64 -- hop it to partition 0 of a PSUM tile
                # via a K=1 matmul so partition_broadcast can read it.
                dcp = smol.tile([65, QCH], BF16, tag="dcp", name=f"dcp{nm}")
                nc.vector.tensor_copy(dcp[64:65, :], psO0[64:65, :])
                rb1 = smol.tile([1, QCH], F32, tag="rb1", name=f"rb1{nm}")
                nc.vector.reciprocal_approx_fast(rb1[:], psO1[0:1, :])
                sden = pso.tile([1, QCH], F32, tag="o0", name=f"sden{nm}")
                nc.tensor.matmul(sden[:], sel[64:65, :], dcp[64:65, :],
                                 start=True, stop=True)
                rb0 = smol.tile([1, QCH], F32, tag="rb0", name=f"rb0{nm}")
                nc.vector.reciprocal_approx_fast(rb0[:], sden[:])
                bc1 = smol.tile([128, QCH], F32, tag="bc1", name=f"bc1{nm}")
                nc.gpsimd.partition_broadcast(bc1[:], rb1[:])
                bc0 = smol.tile([128, QCH], F32, tag="bc0", name=f"bc0{nm}")
                nc.gpsimd.partition_broadcast(bc0[:], rb0[:])
                nc.vector.tensor_mul(OT[b][64:128, oqs], psO1[64:128, :],
                                     bc1[64:128, :])
                nc.vector.tensor_mul(OT[b][0:64, oqs], psO0[0:64, :],
                                     bc0[0:64, :])

                pending.append(
                    lambda b=b, qc=qc: proj_qc(pso, ybn, b, qc, eng_pick))

        # ---- phases ----
        with tc.tile_pool(name="P1", bufs=2, space="PSUM") as P1, \
             tc.tile_pool(name="P2", bufs=2, space="PSUM") as P2, \
             tc.tile_pool(name="swp", bufs=2) as swpp, \
             tc.tile_pool(name="rtmp", bufs=2) as rtp, \
             tc.tile_pool(name="ptp", bufs=6) as ptp, \
             tc.tile_pool(name="mbp", bufs=4) as mbp, \
             tc.tile_pool(name="smol", bufs=2) as smol, \
             tc.tile_pool(name="ybp", bufs=4) as ybp:
            pending = []
            psy_n = [0]

            def eng_pick(tt, ncol):
                psy_n[0] += 1
                return "scalar" if psy_n[0] % 6 == 5 else "vector"

            # minimal prereqs for attn(b0) qc0: q/k of tokens 0:1024
            # roped + V tiles 0:4; everything else streams in as filler.
            qk_unit(P1, 0, 1, 0)
            qk_unit(P1, 0, 1, 1)
            rope_p(swpp, rtp, qT, 0, 0, "q00")
            rope_p(swpp, rtp, kT, 0, 0, "k00")
            # b1's inputs + wo stream in later: the marker memsets gate
            # the DMAs (WAR) so they don't steal HBM bandwidth from the
            # critical lead-in tiles
            nc.vector.memset(xq[2][0:1, 0, 0:1], 0.0)
            nc.vector.memset(xq[3][0:1, 0, 0:1], 0.0)
            nc.vector.memset(wo_sb[0:1, 0:1], 0.0)
            nc.sync.dma_start(
                xq[2][:],
                xT[:, 2048:3072].rearrange("(a p) m -> p a m", p=128))
            nc.sync.dma_start(
                xq[3][:],
                xT[:, 3072:4096].rearrange("(a p) m -> p a m", p=128))
            nc.sync.dma_start(wo_sb[:], woT[:])
            v_unit(P2, 0, 0, 4, "scalar")

            # deferred work flushed inside attn(b0)'s kt loop: rest of
            # b0's qkv/rope, then b1's qkv + rope
            filler = [
                lambda: qk_unit(P1, 2, 3, 0),
                lambda: v_unit(P2, 0, 4, 4, "scalar"),
                lambda: qk_unit(P1, 2, 3, 1),
                lambda: rope_p(swpp, rtp, qT, 0, 1, "q01"),
                lambda: rope_p(swpp, rtp, kT, 0, 1, "k01"),
                lambda: v_unit(P2, 0, 8, 4, "vector"),
                lambda: v_unit(P2, 0, 12, 4, "vector"),
                lambda: qk_unit(P1, 4, 5, 0),
                lambda: qk_unit(P1, 4, 5, 1),
                lambda: rope_p(swpp, rtp, qT, 1, 0, "q10"),
                lambda: rope_p(swpp, rtp, kT, 1, 0, "k10"),
                lambda: qk_unit(P1, 6, 7, 0),
                lambda: qk_unit(P1, 6, 7, 1),
                lambda: rope_p(swpp, rtp, qT, 1, 1, "q11"),
                lambda: rope_p(swpp, rtp, kT, 1, 1, "k11"),
            ]

            attn_b((P1, P2, ptp, mbp, smol), 0, ybp, pending, filler,
                   eng_pick)
            # b1's V units flush inside attn(b1): tiles 0-7 are needed
            # from qc0/qc1, emit them first
            filler2 = [
                lambda u=u: v_unit(P2, 1, 4 * u, 4, "vector")
                for u in range(4)
            ]
            for f in filler:
                f()
            attn_b((P1, P2, ptp, mbp, smol), 1, ybp, pending, filler2,
                   eng_pick)
            for f in filler2:
                f()
            for p in pending:
                p()


def _build_program(mode, hasb):
    key = (mode, hasb)
    if key in _PROG_CACHE:
        return _PROG_CACHE[key]
    nc = bacc.Bacc("TRN2", target_bir_lowering=False, debug=False,
                   num_devices=NCORES)
    dram = {
        "xT": nc.dram_tensor("xT", [C, TB], BF16, kind="ExternalInput").ap(),
        "wT": nc.dram_tensor("wT", [C, 384], BF16, kind="ExternalInput").ap(),
        "bqk": nc.dram_tensor("bqk", [128, 2], F32, kind="ExternalInput").ap(),
        "cosT": nc.dram_tensor("cosT", [128, T], BF16,
                               kind="ExternalInput").ap(),
        "sinS": nc.dram_tensor("sinS", [128, T], BF16,
                               kind="ExternalInput").ap(),
        "woT": nc.dram_tensor("woT", [128, C], BF16,
                              kind="ExternalInput").ap(),
        "y": nc.dram_tensor("y", [TB, C], BF16, kind="ExternalOutput").ap(),
    }
    if mode == "bias":
        dram["maskT"] = nc.dram_tensor("maskT", [T, T], BF16,
                                       kind="ExternalInput").ap()
    with tile.TileContext(nc) as tc:
        _emit(tc, mode, hasb, dram)
    nc.compile()
    _PROG_CACHE[key] = (nc, dram)
    return nc, dram


def _rope_tables():
    inv_freq = 1.0 / (10000.0 ** (np.arange(0, HD, 2, dtype=np.float64) / HD))
    freqs = np.arange(T, dtype=np.float64)[:, None] * inv_freq[None, :]
    cos = np.concatenate([np.cos(freqs), np.cos(freqs)], axis=-1)  # [T, 64]
    sin = np.concatenate([np.sin(freqs), np.sin(freqs)], axis=-1)
    cE = cos[:, 0::2].T  # [32, T] rows i -> dim 2i
    cO = cos[:, 1::2].T
    sE = sin[:, 0::2].T
    sO = sin[:, 1::2].T
    cosT = np.concatenate([cE, cO, cE, cO], axis=0)
    sinS = np.concatenate([-sE, sO, -sE, sO], axis=0)
    return (np.ascontiguousarray(cosT.astype(NPBF16)),
            np.ascontiguousarray(sinS.astype(NPBF16)))


def _detect_mode(mask):
    mb = mask.reshape(T, T)
    if np.array_equal(mb != 0, np.tril(np.ones((T, T), dtype=bool))):
        return "causal"
    if np.all(mb != 0):
        return "dense"
    return "bias"


def _prepare_in_maps(x, mask, Wqkv, bqkv, Wo, mode):
    xTn = np.ascontiguousarray(
        x.reshape(TB, C).T.astype(NPBF16))
    cosT, sinS = _rope_tables()
    scale = 1.0 / np.sqrt(np.float32(HD))

    evens = np.arange(0, HD, 2)
    odds = evens + 1

    in_maps = []
    for c in range(NCORES):
        h0, h1 = 2 * c, 2 * c + 1
        qrows = np.concatenate([h0 * HD + evens, h0 * HD + odds,
                                h1 * HD + evens, h1 * HD + odds])
        krows = C + qrows
        vrows = np.concatenate([2 * C + h0 * HD + np.arange(HD),
                                2 * C + h1 * HD + np.arange(HD)])
        wq = Wqkv[qrows, :] * scale
        wk = Wqkv[krows, :]
        wv = Wqkv[vrows, :]
        wT = np.ascontiguousarray(
            np.concatenate([wq, wk, wv], axis=0).T.astype(NPBF16))
        bqk = np.stack([bqkv[qrows] * scale, bqkv[krows]], axis=1)
        woT = np.ascontiguousarray(
            Wo[:, 128 * c:128 * (c + 1)].T.astype(NPBF16))
        im = {
            "xT": xTn, "wT": wT,
            "bqk": np.ascontiguousarray(bqk, dtype=np.float32),
            "cosT": cosT, "sinS": sinS, "woT": woT,
        }
        if mode == "bias":
            im["maskT"] = np.ascontiguousarray(
                (mask.reshape(T, T) != 0).T.astype(NPBF16))
        in_maps.append(im)
    return in_maps


def kernel(x, mask, Wqkv, bqkv, Wo, bo):
    x = np.asarray(x, dtype=np.float32)
    mask = np.asarray(mask)
    Wqkv = np.asarray(Wqkv, dtype=np.float32)
    bqkv = np.asarray(bqkv, dtype=np.float32)
    Wo = np.asarray(Wo, dtype=np.float32)
    bo = np.asarray(bo, dtype=np.float32)

    mode = _detect_mode(mask)
    hasb = bool(np.any(bqkv[:2 * C] != 0.0))
    nc, dram = _build_program(mode, hasb)
    in_maps = _prepare_in_maps(x, mask, Wqkv, bqkv, Wo, mode)

    res = run_bass_kernel_spmd(nc, in_maps, core_ids=list(range(NCORES)))
    y = np.zeros((TB, C), dtype=np.float32)
    for c in range(NCORES):
        y += res.results[c]["y"].astype(np.float32)
    bv = bqkv[2 * C:3 * C]
    y += (bo + bv @ Wo.T)[None, :]
    return y.reshape(B, T, C)


# revision 19
# speedup vs baseline: 1.0708x; 1.0090x over previous
"""Trainium2 Bass kernel for CarlosSelfAttention (B=2, T=2048, C=1024, H=16).

Sharding: tensor-parallel over heads. 8 cores x 2 heads each.
Each core computes q/k/v projections for its 2 heads, RoPE, causal
attention, and a partial out-projection against its 128 columns of Wo.
The host sums the 8 partial outputs (the TP all-reduce) and adds the
output bias plus the (v-bias @ Wo.T) correction term.

v4: keep every engine busy through the phase boundaries.
  - PE pre-warm: junk matmuls during the initial input DMA so HAM is
    at K=8/8 when the first real matmul issues; exp table pre-loaded
    by a dummy ACTIVATE at t=0.
  - phase order: qkv(b0) -> [attn(b0) with deferred qkv(b1)+rope(b1)
    +v(b1) units flushed into the kt loop] -> attn(b1). The PE never
    sees a >1us idle gap, so HAM never re-throttles.
  - scalar engine carries only exp during attention; v-projection
    evictions go to scalar only in the qkv phase (where it is idle);
    diag-mask muls and rope(b1) run on gpsimd; psy evictions on DVE.
  - normalization without DMAs: psO_h0 keeps sum(exp) in row 64,
    psO_h1 gets it in row 0 (128-wide stationary: ones col 0, dims in
    cols 64:128), so both heads' reciprocals broadcast from their own
    psum rows into a full [128, q] tile and the muls write OT's two
    partition halves directly (no rz / otmp partition-move DMAs).
"""

import numpy as np
import ml_dtypes

import concourse.bass as bass
import concourse.tile as tile
from concourse import bacc, mybir
from concourse.bass_utils import run_bass_kernel_spmd

F32 = mybir.dt.float32
BF16 = mybir.dt.bfloat16
AF = mybir.ActivationFunctionType
NPBF16 = ml_dtypes.bfloat16

B, T, C, H, HD = 2, 2048, 1024, 16, 64
NCORES = 8
TB = B * T          # 4096
QCH = 512           # q-chunk (moving dim)
NQC = T // QCH      # 4 q-chunks per batch
NKT = T // 128      # 16 k-tiles per batch
NCT = C // 128      # 8 contraction tiles
NWARM = 10          # junk matmuls to pre-warm the PE clock gate

_PROG_CACHE: dict = {}


def _emit(tc, mode, hasb, dram):
    nc = tc.nc
    from contextlib import ExitStack

    wT, bqk, cosT, sinS, woT, y = (
        dram["wT"], dram["bqk"], dram["cosT"], dram["sinS"],
        dram["woT"], dram["y"])
    xT = dram["xT"]
    maskT = dram.get("maskT")

    with ExitStack() as ctx:
        constp = ctx.enter_context(tc.tile_pool(name="const", bufs=1))
        pers = ctx.enter_context(tc.tile_pool(name="pers", bufs=1))

        # ---- PE pre-warm + ACT exp-table preload (overlaps input DMA) ----
        junk = constp.tile([128, 512], BF16)
        nc.vector.memset(junk[:], 0.0)
        jexp = constp.tile([1, 8], F32)
        nc.scalar.activation(jexp[:], junk[0:1, 0:8], AF.Exp)
        with tc.tile_pool(name="warm", bufs=1, space="PSUM") as WP:
            jp = WP.tile([128, 256], F32)
            for _ in range(NWARM):
                nc.tensor.matmul(jp[:], junk[:, 0:128], junk[:, 0:256],
                                 start=True, stop=True)

        # ---- constants, in first-use order ----
        wsb = constp.tile([128, NCT, 384], BF16)
        nc.sync.dma_start(wsb[:, 0, :], wT[0:128, :])
        xq = [constp.tile([128, NCT, 1024], BF16, name=f"xq{i}")
              for i in range(4)]

        def xs(ct, lo, n):
            """x slice [128, n] covering global token cols [lo, lo+n)."""
            q, l = lo // 1024, lo % 1024
            assert l + n <= 1024
            return xq[q][:, ct, l:l + n]

        nc.sync.dma_start(
            wsb[:, 1:NCT, :],
            wT[128:NCT * 128, :].rearrange("(a p) m -> p a m", p=128))
        for ct in range(NCT):
            nc.sync.dma_start(xq[0][:, ct, :],
                              xT[ct * 128:(ct + 1) * 128, 0:1024])
        nc.sync.dma_start(
            xq[1][:], xT[:, 1024:2048].rearrange("(a p) m -> p a m", p=128))
        cos_sb = constp.tile([128, T], BF16)
        nc.sync.dma_start(cos_sb[:], cosT[:])
        sin_sb = constp.tile([128, T], BF16)
        nc.sync.dma_start(sin_sb[:], sinS[:])
        if hasb:
            bqk_sb = constp.tile([128, 2], F32)
            nc.sync.dma_start(bqk_sb[:], bqk[:])
        wo_sb = constp.tile([128, C], BF16)
        # b1's x quarters + wo are loaded later (after b0's qkv units are
        # emitted) so they don't steal HBM bandwidth from the critical
        # first tiles
        # selector column for the h0 denominator hop: a K=1 matmul with
        # this as stationary moves row 64 to partition 0 of a PSUM tile
        sel = constp.tile([128, 1], BF16)
        nc.vector.memset(sel[:], 1.0)
        # triangular causal mask for the diagonal 128-blocks, both heads:
        # [128, 2, 128], keep (1.0) iff col >= partition
        mtri = constp.tile([128, 2, 128], BF16)
        nc.vector.memset(mtri[:], 1.0)
        nc.gpsimd.affine_select(
            out=mtri[:], in_=mtri[:], compare_op=mybir.AluOpType.is_ge,
            fill=0.0, base=0, channel_multiplier=-1,
            pattern=[[0, 2], [1, 128]])

        # ---- persistent activations ----
        qT = pers.tile([128, TB], BF16)
        kT = pers.tile([128, TB], BF16)
        # V for head 0 in [ktok, dim] layout; col 64 of each k-tile stays
        # 1.0 so the P@V matmul also produces sum(exp) in psO row 64.
        # V for head 1 is 128 wide: ones in col 0 (sum(exp) -> psO row 0),
        # zeros in 1:64, dims in 64:128 (lands on OT partitions 64:128).
        V0 = [pers.tile([128, NKT * (HD + 1)], BF16, name=f"V0b{b}")
              for b in range(B)]
        V1 = [pers.tile([128, NKT, 128], BF16, name=f"V1b{b}")
              for b in range(B)]
        for b in range(B):
            nc.vector.memset(V0[b][:], 1.0)
            nc.vector.memset(V1[b][:], 0.0)
            nc.vector.memset(V1[b][:, :, 0:1], 1.0)
        OT = [pers.tile([128, T], BF16, name=f"OTb{b}") for b in range(B)]

        def qk_unit(P1, tca, tcb, g):
            """q (g=0) or k (g=1) projection for two adjacent 512-token
            chunks, shared stationaries, one [128, 1024] PSUM tile."""
            ts2 = slice(tca * QCH, (tcb + 1) * QCH)
            psab = P1.tile([128, 2 * QCH], F32, tag="s",
                           name=f"psq{tca}_{g}")
            for ct in range(NCT):
                w = wsb[:, ct, g * 128:(g + 1) * 128]
                nc.tensor.matmul(psab[:, 0:QCH], w,
                                 xs(ct, tca * QCH, QCH),
                                 start=(ct == 0), stop=(ct == NCT - 1))
                nc.tensor.matmul(psab[:, QCH:2 * QCH], w,
                                 xs(ct, tcb * QCH, QCH),
                                 start=(ct == 0), stop=(ct == NCT - 1))
            dst = qT if g == 0 else kT
            if hasb:
                nc.scalar.activation(dst[:, ts2], psab[:], AF.Identity,
                                     bias=bqk_sb[:, g:g + 1])
            else:
                nc.vector.tensor_copy(dst[:, ts2], psab[:])

        def v_unit(P2, b, tt0, ntt, evict):
            """V tiles [tt0, tt0+ntt) for batch b in [ktok, dim] layout:
            stationary is the x tile, moving is the 128 Wv columns."""
            for tt in range(tt0, tt0 + ntt):
                lo = b * T + tt * 128
                psv = P2.tile([128, 128], F32, tag=f"o{tt % 2}",
                              name=f"psv{b}{tt}")
                for ct in range(NCT):
                    nc.tensor.matmul(psv[:], xs(ct, lo, 128),
                                     wsb[:, ct, 256:384],
                                     start=(ct == 0), stop=(ct == NCT - 1))
                o = tt * (HD + 1)
                if evict == "scalar":
                    nc.scalar.activation(V0[b][:, o:o + HD], psv[:, 0:HD],
                                         AF.Copy)
                    nc.scalar.activation(V1[b][:, tt, 64:128],
                                         psv[:, HD:2 * HD], AF.Copy)
                else:
                    nc.vector.tensor_copy(V0[b][:, o:o + HD], psv[:, 0:HD])
                    nc.vector.tensor_copy(V1[b][:, tt, 64:128],
                                          psv[:, HD:2 * HD])

        def rope_p(swpp, rtp, zt, b, piece, nm):
            """RoPE for one 1024-token piece (chunk pair) of batch b, so
            attention can start before the whole batch is rotated."""
            bs = slice(b * T + piece * 1024, b * T + (piece + 1) * 1024)
            cs = slice(piece * 1024, (piece + 1) * 1024)
            swp = swpp.tile([128, 1024], BF16, tag="swp", name=f"swp{nm}")
            for h in range(2):
                o = h * 64
                nc.sync.dma_start(swp[o:o + 32, :], zt[o + 32:o + 64, bs])
                nc.sync.dma_start(swp[o + 32:o + 64, :], zt[o:o + 32, bs])
            tmp = rtp.tile([128, 1024], BF16, tag="rt", name=f"rt{nm}")
            nc.vector.tensor_mul(tmp[:], swp[:], sin_sb[:, cs])
            nc.vector.tensor_mul(zt[:, bs], zt[:, bs], cos_sb[:, cs])
            nc.vector.tensor_add(zt[:, bs], zt[:, bs], tmp[:])

        def proj_qc(pso, ybn, b, qc, eng_pick):
            """Out-projection units for the 4 token-tiles of (b, qc)."""
            for tt in range(4 * qc, 4 * qc + 4):
                yb = ybn.tile([128, C], BF16, tag="yb", name=f"yb{b}{tt}")
                for ncol in range(2):
                    ps = pso.tile([128, QCH], F32, tag=f"o{ncol}",
                                  name=f"psy{b}{tt}{ncol}")
                    nc.tensor.matmul(
                        ps[:], OT[b][:, tt * 128:(tt + 1) * 128],
                        wo_sb[:, ncol * QCH:(ncol + 1) * QCH],
                        start=True, stop=True)
                    ysl = yb[:, ncol * QCH:(ncol + 1) * QCH]
                    if eng_pick(tt, ncol) == "scalar":
                        nc.scalar.activation(ysl, ps[:], AF.Copy)
                    else:
                        nc.vector.tensor_copy(ysl, ps[:])
                nc.sync.dma_start(
                    y[b * T + tt * 128:b * T + (tt + 1) * 128, :], yb[:])

        def attn_b(pools, b, ybn, pending, filler, eng_pick):
            pss, pso, ptp, mbp, smol = pools
            PIPE = 5
            for qc in range(NQC):
                nk = 4 * (qc + 1) if mode == "causal" else NKT
                psO0 = pso.tile([65, QCH], F32, tag="o0",
                                name=f"psO0_{b}{qc}")
                psO1 = pso.tile([128, QCH], F32, tag="o1",
                                name=f"psO1_{b}{qc}")
                pts = {}

                def offs(kt):
                    if mode == "causal" and kt >= 4 * qc:
                        return (kt - 4 * qc) * 128
                    return 0

                def emit_pv(j, nk=nk, psO0=psO0, psO1=psO1, pts=pts, b=b):
                    st, sp = (j == 0), (j == nk - 1)
                    off = offs(j)
                    pt = pts.pop(j)
                    ptv = pt[:].rearrange("p (h q) -> p h q", q=QCH)
                    nc.tensor.matmul(
                        psO0[:, off:QCH],
                        V0[b][:, j * (HD + 1):(j + 1) * (HD + 1)],
                        ptv[:, 0, off:QCH], start=st, stop=sp)
                    nc.tensor.matmul(
                        psO1[:, off:QCH], V1[b][:, j, :],
                        ptv[:, 1, off:QCH], start=st, stop=sp)

                for kt in range(nk):
                    if kt == 1 and len(pending) >= 2:
                        # flush an out-projection deferred by two q-chunks
                        pending.pop(0)()
                    if kt >= 2 and filler:
                        # flush one deferred unit of the other batch's
                        # qkv/rope work into this kt slot
                        filler.pop(0)()
                    ks = slice(b * T + kt * 128, b * T + (kt + 1) * 128)
                    off = offs(kt)
                    qs = slice(b * T + qc * QCH + off, b * T + (qc + 1) * QCH)
                    psS = pss.tile([128, 2 * QCH], F32, tag="s",
                                   name=f"psS{b}{qc}{kt}")
                    # the two heads occupy disjoint PE row groups
                    # (partitions 0:64 / 64:128) -> emitted back-to-back
                    # they execute concurrently
                    nc.tensor.matmul(psS[:, off:QCH], kT[0:64, ks],
                                     qT[0:64, qs], start=True, stop=True)
                    nc.tensor.matmul(psS[:, QCH + off:2 * QCH],
                                     kT[64:128, ks], qT[64:128, qs],
                                     start=True, stop=True)
                    pt = ptp.tile([128, 2 * QCH], BF16, tag="pt",
                                  name=f"pt{b}{qc}{kt}")
                    psv_ = psS[:].rearrange("p (h q) -> p h q", q=QCH)
                    ptv = pt[:].rearrange("p (h q) -> p h q", q=QCH)
                    nc.scalar.activation(ptv[:, :, off:QCH],
                                         psv_[:, :, off:QCH], AF.Exp)
                    if mode == "causal" and kt >= 4 * qc:
                        nc.vector.tensor_mul(ptv[:, :, off:off + 128],
                                             ptv[:, :, off:off + 128],
                                             mtri[:])
                    elif mode == "bias":
                        mt = mbp.tile([128, QCH], BF16, tag="mb",
                                      name=f"mt{b}{qc}{kt}")
                        nc.sync.dma_start(
                            mt[:], maskT[kt * 128:(kt + 1) * 128,
                                         qc * QCH:(qc + 1) * QCH])
                        nc.vector.tensor_mul(ptv[:, 0, :], ptv[:, 0, :],
                                             mt[:])
                        nc.vector.tensor_mul(ptv[:, 1, :], ptv[:, 1, :],
                                             mt[:])
                    pts[kt] = pt
                    if kt >= PIPE:
                        emit_pv(kt - PIPE)
                for j in range(max(0, nk - PIPE), nk):
                    emit_pv(j)

                # normalize + evict; sum(exp) is in psO0 row 64 / psO1
                # row 0. First hop copies psO out of PSUM so the bank
                # frees for the next q-chunk's P@V; reciprocal of the sum
                # row broadcasts (gpsimd, partition 0 of the src AP) into
                # a full [128, q] tile, and the muls write OT's partition
                # halves in place -- no partition-moving DMAs.
                oqs = slice(qc * QCH, (qc + 1) * QCH)
                nm = f"{b}{qc}"
                # h1's sum(exp) is already at psO1 partition 0; h0's sits
                # at psO0 row 64 -- hop it to partition 0 of a PSUM tile
                # via a K=1 matmul so partition_broadcast can read it.
                dcp = smol.tile([65, QCH], BF16, tag="dcp", name=f"dcp{nm}")
                nc.vector.tensor_copy(dcp[64:65, :], psO0[64:65, :])
                rb1 = smol.tile([1, QCH], F32, tag="rb1", name=f"rb1{nm}")
                nc.vector.reciprocal_approx_fast(rb1[:], psO1[0:1, :])
                sden = pso.tile([1, QCH], F32, tag="o0", name=f"sden{nm}")
                nc.tensor.matmul(sden[:], sel[64:65, :], dcp[64:65, :],
                                 start=True, stop=True)
                rb0 = smol.tile([1, QCH], F32, tag="rb0", name=f"rb0{nm}")
                nc.vector.reciprocal_approx_fast(rb0[:], sden[:])
                bc1 = smol.tile([128, QCH], F32, tag="bc1", name=f"bc1{nm}")
                nc.gpsimd.partition_broadcast(bc1[:], rb1[:])
                bc0 = smol.tile([128, QCH], F32, tag="bc0", name=f"bc0{nm}")
                nc.gpsimd.partition_broadcast(bc0[:], rb0[:])
                nc.vector.tensor_mul(OT[b][64:128, oqs], psO1[64:128, :],
                                     bc1[64:128, :])
                nc.vector.tensor_mul(OT[b][0:64, oqs], psO0[0:64, :],
                                     bc0[0:64, :])

                pending.append(
                    lambda b=b, qc=qc: proj_qc(pso, ybn, b, qc, eng_pick))

        # ---- phases ----
        with tc.tile_pool(name="P1", bufs=2, space="PSUM") as P1, \
             tc.tile_pool(name="P2", bufs=2, space="PSUM") as P2, \
             tc.tile_pool(name="swp", bufs=2) as swpp, \
             tc.tile_pool(name="rtmp", bufs=2) as rtp, \
             tc.tile_pool(name="ptp", bufs=6) as ptp, \
             tc.tile_pool(name="mbp", bufs=4) as mbp, \
             tc.tile_pool(name="smol", bufs=2) as smol, \
             tc.tile_pool(name="ybp", bufs=4) as ybp:
            pending = []
            psy_n = [0]

            def eng_pick(tt, ncol):
                psy_n[0] += 1
                return "scalar" if psy_n[0] % 6 == 5 else "vector"

            # qkv + rope for b0; v evictions ride the idle scalar engine.
            qk_unit(P1, 0, 1, 0)
            qk_unit(P1, 0, 1, 1)
            rope_p(swpp, rtp, qT, 0, 0, "q00")
            rope_p(swpp, rtp, kT, 0, 0, "k00")
            qk_unit(P1, 2, 3, 0)
            qk_unit(P1, 2, 3, 1)
            rope_p(swpp, rtp, qT, 0, 1, "q01")
            rope_p(swpp, rtp, kT, 0, 1, "k01")
            # b1's inputs + wo stream in later: the marker memsets gate
            # the DMAs (WAR) so they don't steal HBM bandwidth from the
            # critical lead-in tiles
            nc.vector.memset(xq[2][0:1, 0, 0:1], 0.0)
            nc.vector.memset(xq[3][0:1, 0, 0:1], 0.0)
            nc.vector.memset(wo_sb[0:1, 0:1], 0.0)
            nc.sync.dma_start(
                xq[2][:],
                xT[:, 2048:3072].rearrange("(a p) m -> p a m", p=128))
            nc.sync.dma_start(
                xq[3][:],
                xT[:, 3072:4096].rearrange("(a p) m -> p a m", p=128))
            nc.sync.dma_start(wo_sb[:], woT[:])
            for u in range(2):
                v_unit(P2, 0, 4 * u, 4, "scalar")

            # deferred work flushed inside attn(b0)'s kt loop: rest of
            # b0's V, then b1's qkv + rope
            filler = [
                lambda: v_unit(P2, 0, 8, 4, "vector"),
                lambda: v_unit(P2, 0, 12, 4, "vector"),
                lambda: qk_unit(P1, 4, 5, 0),
                lambda: qk_unit(P1, 4, 5, 1),
                lambda: rope_p(swpp, rtp, qT, 1, 0, "q10"),
                lambda: rope_p(swpp, rtp, kT, 1, 0, "k10"),
                lambda: qk_unit(P1, 6, 7, 0),
                lambda: qk_unit(P1, 6, 7, 1),
                lambda: rope_p(swpp, rtp, qT, 1, 1, "q11"),
                lambda: rope_p(swpp, rtp, kT, 1, 1, "k11"),
            ]

            attn_b((P1, P2, ptp, mbp, smol), 0, ybp, pending, filler,
                   eng_pick)
            # b1's V units flush inside attn(b1): tiles 0-7 are needed
            # from qc0/qc1, emit them first
            filler2 = [
                lambda u=u: v_unit(P2, 1, 4 * u, 4, "vector")
                for u in range(4)
            ]
            for f in filler:
                f()
            attn_b((P1, P2, ptp, mbp, smol), 1, ybp, pending, filler2,
                   eng_pick)
            for f in filler2:
                f()
            for p in pending:
                p()


def _build_program(mode, hasb):
    key = (mode, hasb)
    if key in _PROG_CACHE:
        return _PROG_CACHE[key]
    nc = bacc.Bacc("TRN2", target_bir_lowering=False, debug=False,
                   num_devices=NCORES)
    dram = {
        "xT": nc.dram_tensor("xT", [C, TB], BF16, kind="ExternalInput").ap(),
        "wT": nc.dram_tensor("wT", [C, 384], BF16, kind="ExternalInput").ap(),
        "bqk": nc.dram_tensor("bqk", [128, 2], F32, kind="ExternalInput").ap(),
        "cosT": nc.dram_tensor("cosT", [128, T], BF16,
                               kind="ExternalInput").ap(),
        "sinS": nc.dram_tensor("sinS", [128, T], BF16,
                               kind="ExternalInput").ap(),
        "woT": nc.dram_tensor("woT", [128, C], BF16,
                              kind="ExternalInput").ap(),
        "y": nc.dram_tensor("y", [TB, C], BF16, kind="ExternalOutput").ap(),
    }
    if mode == "bias":
        dram["maskT"] = nc.dram_tensor("maskT", [T, T], BF16,
                                       kind="ExternalInput").ap()
    with tile.TileContext(nc) as tc:
        _emit(tc, mode, hasb, dram)
    nc.compile()
    _PROG_CACHE[key] = (nc, dram)
    return nc, dram


def _rope_tables():
    inv_freq = 1.0 / (10000.0 ** (np.arange(0, HD, 2, dtype=np.float64) / HD))
    freqs = np.arange(T, dtype=np.float64)[:, None] * inv_freq[None, :]
    cos = np.concatenate([np.cos(freqs), np.cos(freqs)], axis=-1)  # [T, 64]
    sin = np.concatenate([np.sin(freqs), np.sin(freqs)], axis=-1)
    cE = cos[:, 0::2].T  # [32, T] rows i -> dim 2i
    cO = cos[:, 1::2].T
    sE = sin[:, 0::2].T
    sO = sin[:, 1::2].T
    cosT = np.concatenate([cE, cO, cE, cO], axis=0)
    sinS = np.concatenate([-sE, sO, -sE, sO], axis=0)
    return (np.ascontiguousarray(cosT.astype(NPBF16)),
            np.ascontiguousarray(sinS.astype(NPBF16)))


def _detect_mode(mask):
    mb = mask.reshape(T, T)
    if np.array_equal(mb != 0, np.tril(np.ones((T, T), dtype=bool))):
        return "causal"
    if np.all(mb != 0):
        return "dense"
    return "bias"


def _prepare_in_maps(x, mask, Wqkv, bqkv, Wo, mode):
    xTn = np.ascontiguousarray(
        x.reshape(TB, C).T.astype(NPBF16))
    cosT, sinS = _rope_tables()
    scale = 1.0 / np.sqrt(np.float32(HD))

    evens = np.arange(0, HD, 2)
    odds = evens + 1

    in_maps = []
    for c in range(NCORES):
        h0, h1 = 2 * c, 2 * c + 1
        qrows = np.concatenate([h0 * HD + evens, h0 * HD + odds,
                                h1 * HD + evens, h1 * HD + odds])
        krows = C + qrows
        vrows = np.concatenate([2 * C + h0 * HD + np.arange(HD),
                                2 * C + h1 * HD + np.arange(HD)])
        wq = Wqkv[qrows, :] * scale
        wk = Wqkv[krows, :]
        wv = Wqkv[vrows, :]
        wT = np.ascontiguousarray(
            np.concatenate([wq, wk, wv], axis=0).T.astype(NPBF16))
        bqk = np.stack([bqkv[qrows] * scale, bqkv[krows]], axis=1)
        woT = np.ascontiguousarray(
            Wo[:, 128 * c:128 * (c + 1)].T.astype(NPBF16))
        im = {
            "xT": xTn, "wT": wT,
            "bqk": np.ascontiguousarray(bqk, dtype=np.float32),
            "cosT": cosT, "sinS": sinS, "woT": woT,
        }
        if mode == "bias":
            im["maskT"] = np.ascontiguousarray(
                (mask.reshape(T, T) != 0).T.astype(NPBF16))
        in_maps.append(im)
    return in_maps


def kernel(x, mask, Wqkv, bqkv, Wo, bo):
    x = np.asarray(x, dtype=np.float32)
    mask = np.asarray(mask)
    Wqkv = np.asarray(Wqkv, dtype=np.float32)
    bqkv = np.asarray(bqkv, dtype=np.float32)
    Wo = np.asarray(Wo, dtype=np.float32)
    bo = np.asarray(bo, dtype=np.float32)

    mode = _detect_mode(mask)
    hasb = bool(np.any(bqkv[:2 * C] != 0.0))
    nc, dram = _build_program(mode, hasb)
    in_maps = _prepare_in_maps(x, mask, Wqkv, bqkv, Wo, mode)

    res = run_bass_kernel_spmd(nc, in_maps, core_ids=list(range(NCORES)))
    y = np.zeros((TB, C), dtype=np.float32)
    for c in range(NCORES):
        y += res.results[c]["y"].astype(np.float32)
    bv = bqkv[2 * C:3 * C]
    y += (bo + bv @ Wo.T)[None, :]
    return y.reshape(B, T, C)
